# revision 11
# baseline (speedup 1.0000x reference)
"""Trainium2 Bass kernel for CubeFaceNN.

Computes, for x of shape [8, 1, 128, 128, 128] (f32):
    out[b, i, p] = relu(x[b, 0, p] - x[b, 0, p + OFF[i]])   (zero padded)
with OFF = [(0,-1,-1), (-1,0,-1), (1,-1,-1), (-1,1,-1), (-1,-1,0), (-1,-1,1)]
(derived from the reference's adj % 3 - 1 indexing).

Sharding: pure data parallel — batch b -> NeuronCore b (8 cores).

Per-core layout: depth d on the 128 SBUF partitions, (h, w) in the free
dims. x is fully resident in SBUF (64KB/partition); a partition-shifted
copy xp[d] = x[d+1] is loaded straight from HBM in prefetched h-chunks
(compute engines cannot address SBUF at a partition offset of 1).
Channels with od = -1 are computed in the substituted frame
    out[i, d'+1] = relu(xp[d'] - x[d', h+oh, w+ow])
so one shifted copy serves all five d-shifting channels; the d-boundary
faces are written from small [h, w]-layout plane tiles.

DMA rules learned from traces/probes on this silicon:
  - The HWDGE dynamic ring drains through a single SDMA engine
    (~27 GB/s) -> only tiny plane/tail transfers use nc.sync.
  - SWDGE (nc.gpsimd) spreads descriptors across engines only for
    per-partition runs <= 16 KB.
  - Partitions map to SDMA engines via an interleaved port map: [0:64)
    uses the 8 even engines, [64:128) the 8 odd ones. A single
    127/128-partition transfer runs its engines in near-lockstep with
    per-descriptor completion bookkeeping (~110 GB/s); TWO DMAs over
    disjoint halves sustain ~230 GB/s. All big transfers are issued as
    even/odd half-partition pairs.
"""

import numpy as np

import concourse.bacc as bacc
import concourse.mybir as mybir
import concourse.tile as tile
from concourse.bass_utils import run_bass_kernel_spmd

D = H = W = 128
HALF = 64
N_CORES = 8
HC = 16  # compute/store h-chunk
XC = 16  # xp load h-chunk
F32 = mybir.dt.float32

# (od, oh, ow) per output channel
OFFSETS = [(0, -1, -1), (-1, 0, -1), (1, -1, -1), (-1, 1, -1), (-1, -1, 0), (-1, -1, 1)]

_NC_CACHE = {}


def build_nc(debug=False):
    nc = bacc.Bacc("TRN2", target_bir_lowering=False, debug=debug)
    x = nc.dram_tensor("x", [D, H, W], F32, kind="ExternalInput")
    out = nc.dram_tensor("out", [6, D, H, W], F32, kind="ExternalOutput")

    sub = mybir.AluOpType.subtract
    relu = mybir.ActivationFunctionType.Relu
    n_chunks = H // HC

    def split_dma(dst, src, dmax):
        # even-engine half then odd-engine half
        nc.gpsimd.dma_start(out=dst[0:HALF], in_=src[0:HALF])
        nc.gpsimd.dma_start(out=dst[HALF:dmax], in_=src[HALF:dmax])

    with tile.TileContext(nc) as tc:
        with (
            tc.tile_pool(name="xt", bufs=1) as xt_pool,
            tc.tile_pool(name="xp", bufs=2) as xp_pool,
            tc.tile_pool(name="och", bufs=10) as och_pool,
            tc.tile_pool(name="plane", bufs=2) as plane_pool,
        ):
            # x fully resident, loaded as 4 x 2 half-partition chunks
            xt = xt_pool.tile([D, H, W], F32)
            for c in range(H // XC):
                hsl = slice(c * XC, (c + 1) * XC)
                split_dma(xt[:, hsl, :], x[:, hsl, :], D)

            def load_xp_chunk(cx):
                # xp rows [cx*XC - 1, cx*XC + XC) on partitions 0..126
                # (xp[d, r] = x[d+1, lo + r]); 32-row (16 KB) halves +
                # <=1-row tail on the HWDGE ring.
                lo = max(0, cx * XC - 1)
                hi = cx * XC + XC
                t = xp_pool.tile([D, XC + 1, W], F32)
                base = 1 if cx > 0 else 0  # local row of absolute row cx*XC
                nc.gpsimd.dma_start(
                    out=t[0:HALF, 0:XC, :], in_=x[1 : HALF + 1, lo : lo + XC, :]
                )
                nc.gpsimd.dma_start(
                    out=t[HALF : D - 1, 0:XC, :], in_=x[HALF + 1 : D, lo : lo + XC, :]
                )
                if hi - lo > XC:
                    nc.sync.dma_start(
                        out=t[0 : D - 1, XC : XC + 1, :], in_=x[1:D, lo + XC : hi, :]
                    )
                return t, base

            xp_tiles = {0: load_xp_chunk(0)}

            # d-boundary planes: out[i, 0] = relu(x[0]) for od=-1 channels,
            # out[2, 127] = relu(x[127]); h on partitions so relu is wide.
            p0 = plane_pool.tile([H, W], F32)
            nc.sync.dma_start(out=p0[:], in_=x[0])
            nc.vector.tensor_scalar_max(p0[:], p0[:], 0.0)
            for i, (od, _, _) in enumerate(OFFSETS):
                if od == -1:
                    nc.sync.dma_start(out=out[i, 0], in_=p0[:])
            p1 = plane_pool.tile([H, W], F32)
            nc.sync.dma_start(out=p1[:], in_=x[D - 1])
            nc.vector.tensor_scalar_max(p1[:], p1[:], 0.0)
            nc.sync.dma_start(out=out[2, D - 1], in_=p1[:])

            for c in range(n_chunks):
                h0 = c * HC
                cx = h0 // XC  # xp tile covering this compute chunk
                if h0 % XC == 0:
                    xp, xpb = xp_tiles.pop(cx)
                    if cx + 1 < H // XC:  # prefetch one XC block ahead
                        xp_tiles[cx + 1] = load_xp_chunk(cx + 1)
                x0 = cx * XC

                def xprow(h):  # absolute h row -> local xp row
                    return h - x0 + xpb

                for i, (od, oh, ow) in enumerate(OFFSETS):
                    # A = operand aligned with the output partition frame,
                    # S = the d-shifted operand (reads at h+oh, w+ow).
                    dc = D if od == 0 else D - 1

                    hs = max(h0, -oh)
                    he = min(h0 + HC, H - max(0, oh))
                    ws = max(0, -ow)
                    we = W - max(0, ow)

                    if od == -1:  # substituted frame: A=xp, S=xt
                        in0 = xp[0:dc, xprow(hs) : xprow(he), ws:we]
                        in1 = xt[0:dc, hs + oh : he + oh, ws + ow : we + ow]
                    elif od == 1:  # A=xt, S=xp
                        in0 = xt[0:dc, hs:he, ws:we]
                        in1 = xp[
                            0:dc, xprow(hs + oh) : xprow(he + oh), ws + ow : we + ow
                        ]
                    else:
                        in0 = xt[0:dc, hs:he, ws:we]
                        in1 = xt[0:dc, hs + oh : he + oh, ws + ow : we + ow]

                    och = och_pool.tile([D, HC, W], F32)
                    nc.vector.tensor_tensor(
                        out=och[0:dc, hs - h0 : he - h0, ws:we],
                        in0=in0,
                        in1=in1,
                        op=sub,
                    )
                    # boundary strips (shifted source zero there -> relu(A));
                    # on ACT so the store depends on one engine's tail only.
                    def strip_src(hb_s, hb_e, wb_s, wb_e):
                        if od == -1:
                            return xp[0:dc, xprow(hb_s) : xprow(hb_e), wb_s:wb_e]
                        return xt[0:dc, hb_s:hb_e, wb_s:wb_e]

                    if oh == -1 and h0 == 0:
                        nc.scalar.activation(
                            och[0:dc, 0:1, :], strip_src(0, 1, 0, W), relu
                        )
                    if oh == 1 and h0 + HC == H:
                        nc.scalar.activation(
                            och[0:dc, HC - 1 : HC, :], strip_src(H - 1, H, 0, W), relu
                        )
                    if ow != 0:
                        wb = 0 if ow == -1 else W - 1
                        nc.scalar.activation(
                            och[0:dc, hs - h0 : he - h0, wb : wb + 1],
                            strip_src(hs, he, wb, wb + 1),
                            relu,
                        )
                    nc.scalar.activation(
                        och[0:dc, hs - h0 : he - h0, ws:we],
                        och[0:dc, hs - h0 : he - h0, ws:we],
                        relu,
                    )

                    if od == -1:
                        split_dma(out[i, 1:D, h0 : h0 + HC, :], och, D - 1)
                    elif od == 1:
                        split_dma(out[i, 0 : D - 1, h0 : h0 + HC, :], och, D - 1)
                    else:
                        split_dma(out[i, :, h0 : h0 + HC, :], och, D)

    nc.compile()
    return nc


def _get_nc():
    if "nc" not in _NC_CACHE:
        _NC_CACHE["nc"] = build_nc()
    return _NC_CACHE["nc"]


def kernel(x: np.ndarray) -> np.ndarray:
    assert x.shape == (N_CORES, 1, D, H, W), x.shape
    nc = _get_nc()
    in_maps = [{"x": np.ascontiguousarray(x[b, 0], dtype=np.float32)} for b in range(N_CORES)]
    res = run_bass_kernel_spmd(nc, in_maps, core_ids=list(range(N_CORES)))
    return np.stack([r["out"] for r in res.results], axis=0)


# revision 12
# speedup vs baseline: 1.0082x; 1.0082x over previous
"""Trainium2 Bass kernel for CubeFaceNN.

Computes, for x of shape [8, 1, 128, 128, 128] (f32):
    out[b, i, p] = relu(x[b, 0, p] - x[b, 0, p + OFF[i]])   (zero padded)
with OFF = [(0,-1,-1), (-1,0,-1), (1,-1,-1), (-1,1,-1), (-1,-1,0), (-1,-1,1)]
(derived from the reference's adj % 3 - 1 indexing).

Sharding: pure data parallel — batch b -> NeuronCore b (8 cores).

Per-core layout: depth d on the 128 SBUF partitions, (h, w) in the free
dims. x is fully resident in SBUF (64KB/partition); a partition-shifted
copy xp[d] = x[d+1] is loaded straight from HBM in prefetched h-chunks
(compute engines cannot address SBUF at a partition offset of 1).
Channels with od = -1 are computed in the substituted frame
    out[i, d'+1] = relu(xp[d'] - x[d', h+oh, w+ow])
so one shifted copy serves all five d-shifting channels; the d-boundary
faces are written from small [h, w]-layout plane tiles.

DMA rules learned from traces/probes on this silicon:
  - The HWDGE dynamic ring drains through a single SDMA engine
    (~27 GB/s) -> only tiny plane/tail transfers use nc.sync.
  - SWDGE (nc.gpsimd) spreads descriptors across engines only for
    per-partition runs <= 16 KB.
  - Partitions map to SDMA engines via an interleaved port map: [0:64)
    uses the 8 even engines, [64:128) the 8 odd ones. A single
    127/128-partition transfer runs its engines in near-lockstep with
    per-descriptor completion bookkeeping (~110 GB/s); TWO DMAs over
    disjoint halves sustain ~230 GB/s. All big transfers are issued as
    even/odd half-partition pairs.
"""

import numpy as np

import concourse.bacc as bacc
import concourse.mybir as mybir
import concourse.tile as tile
from concourse.bass_utils import run_bass_kernel_spmd

D = H = W = 128
HALF = 64
N_CORES = 8
HC = 16  # compute/store h-chunk
XC = 32  # xp load h-chunk
F32 = mybir.dt.float32

# (od, oh, ow) per output channel
OFFSETS = [(0, -1, -1), (-1, 0, -1), (1, -1, -1), (-1, 1, -1), (-1, -1, 0), (-1, -1, 1)]

_NC_CACHE = {}


def build_nc(debug=False):
    nc = bacc.Bacc("TRN2", target_bir_lowering=False, debug=debug)
    x = nc.dram_tensor("x", [D, H, W], F32, kind="ExternalInput")
    out = nc.dram_tensor("out", [6, D, H, W], F32, kind="ExternalOutput")

    sub = mybir.AluOpType.subtract
    relu = mybir.ActivationFunctionType.Relu
    n_chunks = H // HC

    def split_dma(dst, src, dmax):
        # even-engine half then odd-engine half
        nc.gpsimd.dma_start(out=dst[0:HALF], in_=src[0:HALF])
        nc.gpsimd.dma_start(out=dst[HALF:dmax], in_=src[HALF:dmax])

    with tile.TileContext(nc) as tc:
        with (
            tc.tile_pool(name="xt", bufs=1) as xt_pool,
            tc.tile_pool(name="xp", bufs=2) as xp_pool,
            tc.tile_pool(name="och", bufs=10) as och_pool,
            tc.tile_pool(name="plane", bufs=2) as plane_pool,
        ):
            # x fully resident, loaded as 4 x 2 half-partition chunks
            xt = xt_pool.tile([D, H, W], F32)
            for c in range(H // XC):
                hsl = slice(c * XC, (c + 1) * XC)
                split_dma(xt[:, hsl, :], x[:, hsl, :], D)

            def load_xp_chunk(cx):
                # xp rows [cx*XC - 1, cx*XC + XC) on partitions 0..126
                # (xp[d, r] = x[d+1, lo + r]); 32-row (16 KB) halves +
                # <=1-row tail on the HWDGE ring.
                lo = max(0, cx * XC - 1)
                hi = cx * XC + XC
                t = xp_pool.tile([D, XC + 1, W], F32)
                base = 1 if cx > 0 else 0  # local row of absolute row cx*XC
                nc.gpsimd.dma_start(
                    out=t[0:HALF, 0:XC, :], in_=x[1 : HALF + 1, lo : lo + XC, :]
                )
                nc.gpsimd.dma_start(
                    out=t[HALF : D - 1, 0:XC, :], in_=x[HALF + 1 : D, lo : lo + XC, :]
                )
                if hi - lo > XC:
                    nc.sync.dma_start(
                        out=t[0 : D - 1, XC : XC + 1, :], in_=x[1:D, lo + XC : hi, :]
                    )
                return t, base

            xp_tiles = {0: load_xp_chunk(0)}

            # d-boundary planes: out[i, 0] = relu(x[0]) for od=-1 channels,
            # out[2, 127] = relu(x[127]); h on partitions so relu is wide.
            p0 = plane_pool.tile([H, W], F32)
            nc.sync.dma_start(out=p0[:], in_=x[0])
            nc.vector.tensor_scalar_max(p0[:], p0[:], 0.0)
            for i, (od, _, _) in enumerate(OFFSETS):
                if od == -1:
                    nc.sync.dma_start(out=out[i, 0], in_=p0[:])
            p1 = plane_pool.tile([H, W], F32)
            nc.sync.dma_start(out=p1[:], in_=x[D - 1])
            nc.vector.tensor_scalar_max(p1[:], p1[:], 0.0)
            nc.sync.dma_start(out=out[2, D - 1], in_=p1[:])

            for c in range(n_chunks):
                h0 = c * HC
                cx = h0 // XC  # xp tile covering this compute chunk
                if h0 % XC == 0:
                    xp, xpb = xp_tiles.pop(cx)
                    if cx + 1 < H // XC:  # prefetch one XC block ahead
                        xp_tiles[cx + 1] = load_xp_chunk(cx + 1)
                x0 = cx * XC

                def xprow(h):  # absolute h row -> local xp row
                    return h - x0 + xpb

                for i, (od, oh, ow) in enumerate(OFFSETS):
                    # A = operand aligned with the output partition frame,
                    # S = the d-shifted operand (reads at h+oh, w+ow).
                    dc = D if od == 0 else D - 1

                    hs = max(h0, -oh)
                    he = min(h0 + HC, H - max(0, oh))
                    ws = max(0, -ow)
                    we = W - max(0, ow)

                    if od == -1:  # substituted frame: A=xp, S=xt
                        in0 = xp[0:dc, xprow(hs) : xprow(he), ws:we]
                        in1 = xt[0:dc, hs + oh : he + oh, ws + ow : we + ow]
                    elif od == 1:  # A=xt, S=xp
                        in0 = xt[0:dc, hs:he, ws:we]
                        in1 = xp[
                            0:dc, xprow(hs + oh) : xprow(he + oh), ws + ow : we + ow
                        ]
                    else:
                        in0 = xt[0:dc, hs:he, ws:we]
                        in1 = xt[0:dc, hs + oh : he + oh, ws + ow : we + ow]

                    och = och_pool.tile([D, HC, W], F32)
                    nc.vector.tensor_tensor(
                        out=och[0:dc, hs - h0 : he - h0, ws:we],
                        in0=in0,
                        in1=in1,
                        op=sub,
                    )
                    # boundary strips (shifted source zero there -> relu(A));
                    # on ACT so the store depends on one engine's tail only.
                    def strip_src(hb_s, hb_e, wb_s, wb_e):
                        if od == -1:
                            return xp[0:dc, xprow(hb_s) : xprow(hb_e), wb_s:wb_e]
                        return xt[0:dc, hb_s:hb_e, wb_s:wb_e]

                    if oh == -1 and h0 == 0:
                        nc.scalar.activation(
                            och[0:dc, 0:1, :], strip_src(0, 1, 0, W), relu
                        )
                    if oh == 1 and h0 + HC == H:
                        nc.scalar.activation(
                            och[0:dc, HC - 1 : HC, :], strip_src(H - 1, H, 0, W), relu
                        )
                    if ow != 0:
                        wb = 0 if ow == -1 else W - 1
                        nc.scalar.activation(
                            och[0:dc, hs - h0 : he - h0, wb : wb + 1],
                            strip_src(hs, he, wb, wb + 1),
                            relu,
                        )
                    nc.scalar.activation(
                        och[0:dc, hs - h0 : he - h0, ws:we],
                        och[0:dc, hs - h0 : he - h0, ws:we],
                        relu,
                    )

                    if od == -1:
                        split_dma(out[i, 1:D, h0 : h0 + HC, :], och, D - 1)
                    elif od == 1:
                        split_dma(out[i, 0 : D - 1, h0 : h0 + HC, :], och, D - 1)
                    else:
                        split_dma(out[i, :, h0 : h0 + HC, :], och, D)

    nc.compile()
    return nc


def _get_nc():
    if "nc" not in _NC_CACHE:
        _NC_CACHE["nc"] = build_nc()
    return _NC_CACHE["nc"]


def kernel(x: np.ndarray) -> np.ndarray:
    assert x.shape == (N_CORES, 1, D, H, W), x.shape
    nc = _get_nc()
    in_maps = [{"x": np.ascontiguousarray(x[b, 0], dtype=np.float32)} for b in range(N_CORES)]
    res = run_bass_kernel_spmd(nc, in_maps, core_ids=list(range(N_CORES)))
    return np.stack([r["out"] for r in res.results], axis=0)


# revision 13
# speedup vs baseline: 1.0101x; 1.0019x over previous
"""Trainium2 Bass kernel for CubeFaceNN.

Computes, for x of shape [8, 1, 128, 128, 128] (f32):
    out[b, i, p] = relu(x[b, 0, p] - x[b, 0, p + OFF[i]])   (zero padded)
with OFF = [(0,-1,-1), (-1,0,-1), (1,-1,-1), (-1,1,-1), (-1,-1,0), (-1,-1,1)]
(derived from the reference's adj % 3 - 1 indexing).

Sharding: pure data parallel — batch b -> NeuronCore b (8 cores).

Per-core layout: depth d on the 128 SBUF partitions, (h, w) in the free
dims. x is fully resident in SBUF (64KB/partition); a partition-shifted
copy xp[d] = x[d+1] is loaded straight from HBM in prefetched h-chunks
(compute engines cannot address SBUF at a partition offset of 1).
Channels with od = -1 are computed in the substituted frame
    out[i, d'+1] = relu(xp[d'] - x[d', h+oh, w+ow])
so one shifted copy serves all five d-shifting channels; the d-boundary
faces are written from small [h, w]-layout plane tiles.

DMA rules learned from traces/probes on this silicon:
  - The HWDGE dynamic ring drains through a single SDMA engine
    (~27 GB/s) -> only tiny plane/tail transfers use nc.sync.
  - SWDGE (nc.gpsimd) spreads descriptors across engines only for
    per-partition runs <= 16 KB.
  - Partitions map to SDMA engines via an interleaved port map: [0:64)
    uses the 8 even engines, [64:128) the 8 odd ones. A single
    127/128-partition transfer runs its engines in near-lockstep with
    per-descriptor completion bookkeeping (~110 GB/s); TWO DMAs over
    disjoint halves sustain ~230 GB/s. All big transfers are issued as
    even/odd half-partition pairs.
"""

import numpy as np

import concourse.bacc as bacc
import concourse.mybir as mybir
import concourse.tile as tile
from concourse.bass_utils import run_bass_kernel_spmd

D = H = W = 128
HALF = 64
N_CORES = 8
HC = 16  # compute/store h-chunk
XC = 32  # xp load h-chunk
F32 = mybir.dt.float32

# (od, oh, ow) per output channel
OFFSETS = [(0, -1, -1), (-1, 0, -1), (1, -1, -1), (-1, 1, -1), (-1, -1, 0), (-1, -1, 1)]

_NC_CACHE = {}


def build_nc(debug=False):
    nc = bacc.Bacc("TRN2", target_bir_lowering=False, debug=debug)
    x = nc.dram_tensor("x", [D, H, W], F32, kind="ExternalInput")
    out = nc.dram_tensor("out", [6, D, H, W], F32, kind="ExternalOutput")

    sub = mybir.AluOpType.subtract
    relu = mybir.ActivationFunctionType.Relu
    n_chunks = H // HC

    def split_dma(dst, src, dmax):
        # even-engine half then odd-engine half
        nc.gpsimd.dma_start(out=dst[0:HALF], in_=src[0:HALF])
        nc.gpsimd.dma_start(out=dst[HALF:dmax], in_=src[HALF:dmax])

    with tile.TileContext(nc) as tc:
        with (
            tc.tile_pool(name="xt", bufs=1) as xt_pool,
            tc.tile_pool(name="xp", bufs=2) as xp_pool,
            tc.tile_pool(name="och", bufs=8) as och_pool,
            tc.tile_pool(name="plane", bufs=2) as plane_pool,
        ):
            # x fully resident, loaded as 4 x 2 half-partition chunks
            xt = xt_pool.tile([D, H, W], F32)
            for c in range(H // XC):
                hsl = slice(c * XC, (c + 1) * XC)
                split_dma(xt[:, hsl, :], x[:, hsl, :], D)

            def load_xp_chunk(cx):
                # xp rows [cx*XC - 1, cx*XC + XC) on partitions 0..126
                # (xp[d, r] = x[d+1, lo + r]); 32-row (16 KB) halves +
                # <=1-row tail on the HWDGE ring.
                lo = max(0, cx * XC - 1)
                hi = cx * XC + XC
                t = xp_pool.tile([D, XC + 1, W], F32)
                base = 1 if cx > 0 else 0  # local row of absolute row cx*XC
                nc.gpsimd.dma_start(
                    out=t[0:HALF, 0:XC, :], in_=x[1 : HALF + 1, lo : lo + XC, :]
                )
                nc.gpsimd.dma_start(
                    out=t[HALF : D - 1, 0:XC, :], in_=x[HALF + 1 : D, lo : lo + XC, :]
                )
                if hi - lo > XC:
                    nc.sync.dma_start(
                        out=t[0 : D - 1, XC : XC + 1, :], in_=x[1:D, lo + XC : hi, :]
                    )
                return t, base

            xp_tiles = {0: load_xp_chunk(0)}

            # d-boundary planes: out[i, 0] = relu(x[0]) for od=-1 channels,
            # out[2, 127] = relu(x[127]); h on partitions so relu is wide.
            p0 = plane_pool.tile([H, W], F32)
            nc.sync.dma_start(out=p0[:], in_=x[0])
            nc.vector.tensor_scalar_max(p0[:], p0[:], 0.0)
            for i, (od, _, _) in enumerate(OFFSETS):
                if od == -1:
                    nc.sync.dma_start(out=out[i, 0], in_=p0[:])
            p1 = plane_pool.tile([H, W], F32)
            nc.sync.dma_start(out=p1[:], in_=x[D - 1])
            nc.vector.tensor_scalar_max(p1[:], p1[:], 0.0)
            nc.sync.dma_start(out=out[2, D - 1], in_=p1[:])

            for c in range(n_chunks):
                h0 = c * HC
                cx = h0 // XC  # xp tile covering this compute chunk
                if h0 % XC == 0:
                    xp, xpb = xp_tiles.pop(cx)
                    if cx + 1 < H // XC:  # prefetch one XC block ahead
                        xp_tiles[cx + 1] = load_xp_chunk(cx + 1)
                x0 = cx * XC

                def xprow(h):  # absolute h row -> local xp row
                    return h - x0 + xpb

                for i, (od, oh, ow) in enumerate(OFFSETS):
                    # A = operand aligned with the output partition frame,
                    # S = the d-shifted operand (reads at h+oh, w+ow).
                    dc = D if od == 0 else D - 1

                    hs = max(h0, -oh)
                    he = min(h0 + HC, H - max(0, oh))
                    ws = max(0, -ow)
                    we = W - max(0, ow)

                    if od == -1:  # substituted frame: A=xp, S=xt
                        in0 = xp[0:dc, xprow(hs) : xprow(he), ws:we]
                        in1 = xt[0:dc, hs + oh : he + oh, ws + ow : we + ow]
                    elif od == 1:  # A=xt, S=xp
                        in0 = xt[0:dc, hs:he, ws:we]
                        in1 = xp[
                            0:dc, xprow(hs + oh) : xprow(he + oh), ws + ow : we + ow
                        ]
                    else:
                        in0 = xt[0:dc, hs:he, ws:we]
                        in1 = xt[0:dc, hs + oh : he + oh, ws + ow : we + ow]

                    och = och_pool.tile([D, HC, W], F32)
                    nc.vector.tensor_tensor(
                        out=och[0:dc, hs - h0 : he - h0, ws:we],
                        in0=in0,
                        in1=in1,
                        op=sub,
                    )
                    # boundary strips (shifted source zero there -> relu(A));
                    # on ACT so the store depends on one engine's tail only.
                    def strip_src(hb_s, hb_e, wb_s, wb_e):
                        if od == -1:
                            return xp[0:dc, xprow(hb_s) : xprow(hb_e), wb_s:wb_e]
                        return xt[0:dc, hb_s:hb_e, wb_s:wb_e]

                    if oh == -1 and h0 == 0:
                        nc.scalar.activation(
                            och[0:dc, 0:1, :], strip_src(0, 1, 0, W), relu
                        )
                    if oh == 1 and h0 + HC == H:
                        nc.scalar.activation(
                            och[0:dc, HC - 1 : HC, :], strip_src(H - 1, H, 0, W), relu
                        )
                    if ow != 0:
                        wb = 0 if ow == -1 else W - 1
                        nc.scalar.activation(
                            och[0:dc, hs - h0 : he - h0, wb : wb + 1],
                            strip_src(hs, he, wb, wb + 1),
                            relu,
                        )
                    nc.scalar.activation(
                        och[0:dc, hs - h0 : he - h0, ws:we],
                        och[0:dc, hs - h0 : he - h0, ws:we],
                        relu,
                    )

                    if od == -1:
                        split_dma(out[i, 1:D, h0 : h0 + HC, :], och, D - 1)
                    elif od == 1:
                        split_dma(out[i, 0 : D - 1, h0 : h0 + HC, :], och, D - 1)
                    else:
                        split_dma(out[i, :, h0 : h0 + HC, :], och, D)

    nc.compile()
    return nc


def _get_nc():
    if "nc" not in _NC_CACHE:
        _NC_CACHE["nc"] = build_nc()
    return _NC_CACHE["nc"]


def kernel(x: np.ndarray) -> np.ndarray:
    assert x.shape == (N_CORES, 1, D, H, W), x.shape
    nc = _get_nc()
    in_maps = [{"x": np.ascontiguousarray(x[b, 0], dtype=np.float32)} for b in range(N_CORES)]
    res = run_bass_kernel_spmd(nc, in_maps, core_ids=list(range(N_CORES)))
    return np.stack([r["out"] for r in res.results], axis=0)
